# revision 6
# baseline (speedup 1.0000x reference)
"""GatedGraphConv GNN on 8 trn2 cores via Bass/Tile.

Sharding: nodes (and their in-edges) sharded by dst across 8 cores.
Per layer: t = h @ W computed shard-wise (feature-major h in SBUF),
AllGather t -> t_full [100352, 256] DRAM; per-core message pass:
dma_gather(t_full[src]) + one-hot matmul segment-sum into per-block
PSUM; GRU in feature-major; readout MLP + batch-one-hot pooling;
AllReduce pooled [128, 64]; sigmoid head.
"""
import os
import numpy as np

import concourse.bacc as bacc
import concourse.tile as tile
from concourse import mybir
from concourse.bass_utils import run_bass_kernel_spmd

P = 128
N = 100000
NE = 1600000
EFIN = 100
D = 200
L = 6
G = 128
NC = 8
NPC_REAL = N // NC          # 12500
NB = (NPC_REAL + P - 1) // P  # 98
NPC = NB * P                # 12544
NTOT = NC * NPC             # 100352
EF = 256                    # padded row width of t (gather elem, bytes%256==0)
NQ = 4                      # src chunks
QCH = NTOT // NQ            # 25088 rows per chunk (< 32768, int16-safe)
f32 = mybir.dt.float32
bf16 = mybir.dt.bfloat16
i16 = mybir.dt.int16


def _build(QW):
    BLKW = NQ * QW              # windows per block
    WTOT = NB * BLKW            # windows per core
    EPAD = WTOT * P             # padded edges per core
    nc = bacc.Bacc(None, target_bir_lowering=False)

    # ---- parameters (per-core values supplied via in_maps) ----
    h0 = nc.declare_dram_parameter("h0", [D, NPC], f32, isOutput=False)
    xfm = nc.declare_dram_parameter("xfm", [EFIN, NPC], f32, isOutput=False)
    eidx = nc.declare_dram_parameter("eidx", [P, EPAD // 16], i16, isOutput=False)
    edst = nc.declare_dram_parameter("edst", [P, WTOT], f32, isOutput=False)
    bat = nc.declare_dram_parameter("bat", [P, NB], f32, isOutput=False)
    wg = nc.declare_dram_parameter("wg", [L, D, D], f32, isOutput=False)
    wiht = nc.declare_dram_parameter("wiht", [D, 3 * D], f32, isOutput=False)
    whht = nc.declare_dram_parameter("whht", [D, 3 * D], f32, isOutput=False)
    bias4 = nc.declare_dram_parameter("bias4", [D, 4], f32, isOutput=False)  # br,bz,bin,bhn
    w1t = nc.declare_dram_parameter("w1t", [D + EFIN, 256], f32, isOutput=False)
    b1 = nc.declare_dram_parameter("b1", [P, 2], f32, isOutput=False)
    w2t = nc.declare_dram_parameter("w2t", [256, 64], f32, isOutput=False)
    b2 = nc.declare_dram_parameter("b2", [64, 1], f32, isOutput=False)
    outw = nc.declare_dram_parameter("outw", [P, 64], f32, isOutput=False)
    outb = nc.declare_dram_parameter("outb", [P, 1], f32, isOutput=False)
    out = nc.declare_dram_parameter("out", [P, 1], f32, isOutput=True)

    t_c = nc.dram_tensor("t_c", [NPC, EF], bf16)
    t_full = nc.dram_tensor("t_full", [NTOT, EF], bf16, addr_space="Shared")
    pool_c = nc.dram_tensor("pool_c", [P, 64], f32)
    pool_ar = nc.dram_tensor("pool_ar", [P, 64], f32, addr_space="Shared")

    with tile.TileContext(nc) as tc:
        with (
            tc.tile_pool(name="const", bufs=1) as cpool,
            tc.tile_pool(name="persist", bufs=1) as hpool,
        ):
            # persistent feature-major h
            h_lo = hpool.tile([P, NPC], f32)
            h_hi = hpool.tile([72, NPC], f32)
            nc.sync.dma_start(out=h_lo[:], in_=h0[0:P, :])
            nc.sync.dma_start(out=h_hi[:], in_=h0[P:D, :])
            # resident edge metadata
            idx_t = cpool.tile([P, EPAD // 16], i16)
            nc.sync.dma_start(out=idx_t[:], in_=eidx[:])
            dst_t = cpool.tile([P, WTOT], f32)
            nc.sync.dma_start(out=dst_t[:], in_=edst[:])
            bat_t = cpool.tile([P, NB], f32)
            nc.sync.dma_start(out=bat_t[:], in_=bat[:])
            iota_f = cpool.tile([P, QW, P], f32)
            nc.gpsimd.iota(iota_f[:], pattern=[[0, QW], [1, P]], base=0,
                           channel_multiplier=0,
                           allow_small_or_imprecise_dtypes=True)
            from concourse.masks import make_identity
            ident = cpool.tile([P, P], f32)
            make_identity(nc, ident[:])
            # GRU weights (feature-major lhsT layouts)
            wih_lo = cpool.tile([P, 3 * D], f32)
            wih_hi = cpool.tile([72, 3 * D], f32)
            whh_lo = cpool.tile([P, 3 * D], f32)
            whh_hi = cpool.tile([72, 3 * D], f32)
            nc.sync.dma_start(out=wih_lo[:], in_=wiht[0:P, :])
            nc.sync.dma_start(out=wih_hi[:], in_=wiht[P:D, :])
            nc.sync.dma_start(out=whh_lo[:], in_=whht[0:P, :])
            nc.sync.dma_start(out=whh_hi[:], in_=whht[P:D, :])
            bias_lo = cpool.tile([P, 4], f32)
            bias_hi = cpool.tile([72, 4], f32)
            nc.sync.dma_start(out=bias_lo[:], in_=bias4[0:P, :])
            nc.sync.dma_start(out=bias_hi[:], in_=bias4[P:D, :])

            with (
                tc.tile_pool(name="wg", bufs=2) as wgpool,
                tc.tile_pool(name="lay", bufs=2) as lpool,
                tc.tile_pool(name="msg", bufs=2) as mpool,
                tc.tile_pool(name="ps_a", bufs=2, space="PSUM") as ps_a,
                tc.tile_pool(name="ps_t", bufs=2, space="PSUM") as ps_t,
                tc.tile_pool(name="ps_g", bufs=4, space="PSUM") as ps_g,
            ):
                for li in range(L):
                    wg_lo = wgpool.tile([P, D], f32, tag="wglo")
                    wg_hi = wgpool.tile([72, D], f32, tag="wghi")
                    nc.sync.dma_start(out=wg_lo[:], in_=wg[li, 0:P, :])
                    nc.sync.dma_start(out=wg_hi[:], in_=wg[li, P:D, :])
                    # ---- t = h @ W_li, write t_c ----
                    for b in range(NB):
                        cs = slice(b * P, (b + 1) * P)
                        acc = ps_a.tile([P, EF], f32, tag="agg", space="PSUM")
                        nc.tensor.matmul(out=acc[:, 0:D], lhsT=h_lo[:, cs],
                                         rhs=wg_lo[:], start=True, stop=False)
                        nc.tensor.matmul(out=acc[:, 0:D], lhsT=h_hi[:, cs],
                                         rhs=wg_hi[:], start=False, stop=True)
                        t_sb = lpool.tile([P, D], bf16, tag="tsb")
                        nc.scalar.copy(out=t_sb[:], in_=acc[:, 0:D])
                        nc.sync.dma_start(out=t_c[cs, 0:D], in_=t_sb[:])
                    # ---- allgather t ----
                    nc.gpsimd.collective_compute(
                        "AllGather", mybir.AluOpType.bypass,
                        replica_groups=[list(range(NC))],
                        ins=[t_c[:]], outs=[t_full[:]],
                    )
                    # ---- message pass + GRU per block ----
                    for b in range(NB):
                        cs = slice(b * P, (b + 1) * P)
                        acc = ps_a.tile([P, EF], f32, tag="agg", space="PSUM")
                        for q in range(NQ):
                            w0 = (b * NQ + q) * QW
                            msg = mpool.tile([P, QW, EF], bf16, tag="msg")
                            nc.gpsimd.dma_gather(
                                out_ap=msg[:],
                                in_ap=t_full[q * QCH:(q + 1) * QCH, :],
                                idxs_ap=idx_t[:, w0 * 8:(w0 + QW) * 8],
                                num_idxs=QW * P, num_idxs_reg=QW * P,
                                elem_size=EF,
                            )
                            S = lpool.tile([P, QW, P], bf16, tag="sel")
                            nc.vector.tensor_tensor(
                                out=S[:],
                                in0=dst_t[:, w0:w0 + QW].to_broadcast([P, QW, P]),
                                in1=iota_f[:],
                                op=mybir.AluOpType.is_equal,
                            )
                            for wi in range(QW):
                                nc.tensor.matmul(
                                    out=acc[:], lhsT=S[:, wi, :], rhs=msg[:, wi, :],
                                    start=(q == 0 and wi == 0),
                                    stop=(q == NQ - 1 and wi == QW - 1),
                                )
                        # transpose agg -> m feature-major
                        agg_sb = lpool.tile([P, D], f32, tag="aggsb")
                        nc.scalar.copy(out=agg_sb[:], in_=acc[:, 0:D])
                        mT_lo = ps_t.tile([P, P], f32, tag="tr", space="PSUM")
                        mT_hi = ps_t.tile([72, P], f32, tag="tr", space="PSUM")
                        nc.tensor.transpose(out=mT_lo[:], in_=agg_sb[:, 0:P], identity=ident[:])
                        nc.tensor.transpose(out=mT_hi[:], in_=agg_sb[:, P:D], identity=ident[:])
                        m_lo = lpool.tile([P, P], f32, tag="mlo")
                        m_hi = lpool.tile([72, P], f32, tag="mhi")
                        nc.scalar.copy(out=m_lo[:], in_=mT_lo[:])
                        nc.scalar.copy(out=m_hi[:], in_=mT_hi[:])
                        # GRU gates: chunks (0:128)=lo, (128:200)=hi of each gate
                        ps_rz_lo = ps_g.tile([P, 2 * P], f32, tag="gate", space="PSUM")
                        ps_rz_hi = ps_g.tile([72, 2 * P], f32, tag="gate", space="PSUM")
                        ps_n_lo = ps_g.tile([P, 2 * P], f32, tag="gate", space="PSUM")
                        ps_n_hi = ps_g.tile([72, 2 * P], f32, tag="gate", space="PSUM")
                        for (ps_rz, ps_n, g0, gw) in (
                            (ps_rz_lo, ps_n_lo, 0, P),
                            (ps_rz_hi, ps_n_hi, P, 72),
                        ):
                            # r at cols 0:P, z at cols P:2P of ps_rz
                            for (col, gbase) in ((0, 0), (P, D)):
                                nc.tensor.matmul(out=ps_rz[:gw, col:col + P],
                                                 lhsT=wih_lo[:, gbase + g0:gbase + g0 + gw],
                                                 rhs=m_lo[:], start=True, stop=False)
                                nc.tensor.matmul(out=ps_rz[:gw, col:col + P],
                                                 lhsT=wih_hi[:, gbase + g0:gbase + g0 + gw],
                                                 rhs=m_hi[:], start=False, stop=False)
                                nc.tensor.matmul(out=ps_rz[:gw, col:col + P],
                                                 lhsT=whh_lo[:, gbase + g0:gbase + g0 + gw],
                                                 rhs=h_lo[:, cs], start=False, stop=False)
                                nc.tensor.matmul(out=ps_rz[:gw, col:col + P],
                                                 lhsT=whh_hi[:, gbase + g0:gbase + g0 + gw],
                                                 rhs=h_hi[:, cs], start=False, stop=True)
                            # gi_n at cols 0:P, gh_n at cols P:2P of ps_n
                            nc.tensor.matmul(out=ps_n[:gw, 0:P],
                                             lhsT=wih_lo[:, 2 * D + g0:2 * D + g0 + gw],
                                             rhs=m_lo[:], start=True, stop=False)
                            nc.tensor.matmul(out=ps_n[:gw, 0:P],
                                             lhsT=wih_hi[:, 2 * D + g0:2 * D + g0 + gw],
                                             rhs=m_hi[:], start=False, stop=True)
                            nc.tensor.matmul(out=ps_n[:gw, P:2 * P],
                                             lhsT=whh_lo[:, 2 * D + g0:2 * D + g0 + gw],
                                             rhs=h_lo[:, cs], start=True, stop=False)
                            nc.tensor.matmul(out=ps_n[:gw, P:2 * P],
                                             lhsT=whh_hi[:, 2 * D + g0:2 * D + g0 + gw],
                                             rhs=h_hi[:, cs], start=False, stop=True)
                        for (ps_rz, ps_n, bias_t, h_sl, gw) in (
                            (ps_rz_lo, ps_n_lo, bias_lo, h_lo, P),
                            (ps_rz_hi, ps_n_hi, bias_hi, h_hi, 72),
                        ):
                            r_sb = lpool.tile([P, P], f32, tag="r")
                            z_sb = lpool.tile([P, P], f32, tag="z")
                            t1 = lpool.tile([P, P], f32, tag="t1")
                            n_sb = lpool.tile([P, P], f32, tag="n")
                            d_sb = lpool.tile([P, P], f32, tag="d")
                            nc.scalar.activation(out=r_sb[:gw, :], in_=ps_rz[:gw, 0:P],
                                                 func=mybir.ActivationFunctionType.Sigmoid,
                                                 bias=bias_t[:gw, 0:1])
                            nc.scalar.activation(out=z_sb[:gw, :], in_=ps_rz[:gw, P:2 * P],
                                                 func=mybir.ActivationFunctionType.Sigmoid,
                                                 bias=bias_t[:gw, 1:2])
                            # t1 = (gh_n + bhn) * r
                            nc.vector.scalar_tensor_tensor(
                                out=t1[:gw, :], in0=ps_n[:gw, P:2 * P],
                                scalar=bias_t[:gw, 3:4], in1=r_sb[:gw, :],
                                op0=mybir.AluOpType.add, op1=mybir.AluOpType.mult)
                            # n = tanh(gi_n + bin + t1)
                            nc.vector.scalar_tensor_tensor(
                                out=n_sb[:gw, :], in0=ps_n[:gw, 0:P],
                                scalar=bias_t[:gw, 2:3], in1=t1[:gw, :],
                                op0=mybir.AluOpType.add, op1=mybir.AluOpType.add)
                            nc.scalar.activation(out=n_sb[:gw, :], in_=n_sb[:gw, :],
                                                 func=mybir.ActivationFunctionType.Tanh)
                            # h' = n + z * (h - n)
                            nc.vector.tensor_sub(out=d_sb[:gw, :], in0=h_sl[:gw, cs], in1=n_sb[:gw, :])
                            nc.vector.tensor_mul(out=d_sb[:gw, :], in0=d_sb[:gw, :], in1=z_sb[:gw, :])
                            nc.vector.tensor_add(out=h_sl[:gw, cs], in0=n_sb[:gw, :], in1=d_sb[:gw, :])

            # ---- readout ----
            with (
                tc.tile_pool(name="ro", bufs=2) as rpool,
                tc.tile_pool(name="xr", bufs=1) as xpool,
                tc.tile_pool(name="ps_p", bufs=1, space="PSUM") as ps_p,
                tc.tile_pool(name="ps_r", bufs=2, space="PSUM") as ps_r,
            ):
                w1_a = xpool.tile([P, 256], f32)
                w1_b = xpool.tile([72, 256], f32)
                w1_c = xpool.tile([EFIN, 256], f32)
                nc.sync.dma_start(out=w1_a[:], in_=w1t[0:P, :])
                nc.sync.dma_start(out=w1_b[:], in_=w1t[P:D, :])
                nc.sync.dma_start(out=w1_c[:], in_=w1t[D:D + EFIN, :])
                w2_a = xpool.tile([P, 64], f32)
                w2_b = xpool.tile([P, 64], f32)
                nc.sync.dma_start(out=w2_a[:], in_=w2t[0:P, :])
                nc.sync.dma_start(out=w2_b[:], in_=w2t[P:256, :])
                b1_t = xpool.tile([P, 2], f32)
                nc.sync.dma_start(out=b1_t[:], in_=b1[:])
                b2_t = xpool.tile([64, 1], f32)
                nc.sync.dma_start(out=b2_t[:], in_=b2[:])
                outw_t = xpool.tile([P, 64], f32)
                nc.sync.dma_start(out=outw_t[:], in_=outw[:])
                outb_t = xpool.tile([P, 1], f32)
                nc.sync.dma_start(out=outb_t[:], in_=outb[:])

                pooled = ps_p.tile([P, 64], f32, tag="pool", space="PSUM")
                HB = 14
                for b in range(NB):
                    if b % HB == 0:
                        x_t = xpool.tile([EFIN, HB * P], f32, tag="xt")
                        nc.sync.dma_start(
                            out=x_t[:], in_=xfm[:, b * P:(b + HB) * P])
                    cs = slice(b * P, (b + 1) * P)
                    xs = slice((b % HB) * P, (b % HB + 1) * P)
                    ps_f1 = ps_r.tile([P, 256], f32, tag="ro", space="PSUM")
                    for mi in range(2):
                        mslc = slice(mi * P, (mi + 1) * P)
                        nc.tensor.matmul(out=ps_f1[:, mslc], lhsT=w1_a[:, mslc],
                                         rhs=h_lo[:, cs], start=True, stop=False)
                        nc.tensor.matmul(out=ps_f1[:, mslc], lhsT=w1_b[:, mslc],
                                         rhs=h_hi[:, cs], start=False, stop=False)
                        nc.tensor.matmul(out=ps_f1[:, mslc], lhsT=w1_c[:, mslc],
                                         rhs=x_t[:, xs], start=False, stop=True)
                    f1_lo = rpool.tile([P, P], f32, tag="f1lo")
                    f1_hi = rpool.tile([P, P], f32, tag="f1hi")
                    nc.scalar.activation(out=f1_lo[:], in_=ps_f1[:, 0:P],
                                         func=mybir.ActivationFunctionType.Relu,
                                         bias=b1_t[:, 0:1])
                    nc.scalar.activation(out=f1_hi[:], in_=ps_f1[:, P:256],
                                         func=mybir.ActivationFunctionType.Relu,
                                         bias=b1_t[:, 1:2])
                    ps_f2 = ps_r.tile([64, P], f32, tag="ro", space="PSUM")
                    nc.tensor.matmul(out=ps_f2[:], lhsT=w2_a[:, :], rhs=f1_lo[:],
                                     start=True, stop=False)
                    nc.tensor.matmul(out=ps_f2[:], lhsT=w2_b[:, :], rhs=f1_hi[:],
                                     start=False, stop=True)
                    f2_sb = rpool.tile([64, P], f32, tag="f2")
                    nc.scalar.activation(out=f2_sb[:], in_=ps_f2[:],
                                         func=mybir.ActivationFunctionType.Relu,
                                         bias=b2_t[:, 0:1])
                    ps_f2t = ps_r.tile([P, 64], f32, tag="ro", space="PSUM")
                    nc.tensor.transpose(out=ps_f2t[:], in_=f2_sb[:], identity=ident[0:64, 0:64])
                    f2t_sb = rpool.tile([P, 64], f32, tag="f2t")
                    nc.scalar.copy(out=f2t_sb[:], in_=ps_f2t[:])
                    B_sb = rpool.tile([P, P], f32, tag="bsel")
                    nc.vector.tensor_tensor(
                        out=B_sb[:], in0=bat_t[:, b:b + 1].to_broadcast([P, P]),
                        in1=iota_f[:, 0, :], op=mybir.AluOpType.is_equal)
                    nc.tensor.matmul(out=pooled[:], lhsT=B_sb[:], rhs=f2t_sb[:],
                                     start=(b == 0), stop=(b == NB - 1))
                pooled_sb = rpool.tile([P, 64], f32, tag="poolsb")
                nc.scalar.copy(out=pooled_sb[:], in_=pooled[:])
                nc.sync.dma_start(out=pool_c[:], in_=pooled_sb[:])
                nc.gpsimd.collective_compute(
                    "AllReduce", mybir.AluOpType.add,
                    replica_groups=[list(range(NC))],
                    ins=[pool_c[:]], outs=[pool_ar[:]],
                )
                par_sb = rpool.tile([P, 64], f32, tag="par")
                nc.sync.dma_start(out=par_sb[:], in_=pool_ar[:])
                nc.vector.tensor_mul(out=par_sb[:], in0=par_sb[:], in1=outw_t[:])
                red = rpool.tile([P, 1], f32, tag="red")
                nc.vector.reduce_sum(out=red[:], in_=par_sb[:], axis=mybir.AxisListType.X)
                nc.scalar.activation(out=red[:], in_=red[:],
                                     func=mybir.ActivationFunctionType.Sigmoid,
                                     bias=outb_t[:])
                nc.sync.dma_start(out=out[:], in_=red[:])
    nc.finalize()
    return nc


def _prep(x, edge_index, batch):
    """Host-side sharding: returns per-core arrays + QW."""
    src = np.asarray(edge_index[0], np.int64)
    dst = np.asarray(edge_index[1], np.int64)
    batch = np.asarray(batch, np.int64)
    # padded global src ids
    ps_all = (src // NPC_REAL) * NPC + (src % NPC_REAL)
    cores = []
    qcnt = np.zeros((NC, NB, NQ), np.int64)
    for c in range(NC):
        m = (dst >= c * NPC_REAL) & (dst < (c + 1) * NPC_REAL)
        ed = dst[m] - c * NPC_REAL
        es = ps_all[m]
        blk = ed // P
        q = es // QCH
        order = np.lexsort((es, q, ed // P))  # sort by (block, chunk, src)
        cores.append((ed[order], es[order], blk[order], q[order]))
        np.add.at(qcnt[c], (blk[order], q[order]), 1)
    QW = max(1, int(np.ceil(qcnt.max() / P)))
    QW = max(QW, int(os.environ.get("GNN_QW", "0")))
    assert QW <= 16, QW
    BLKW = NQ * QW
    WTOT = NB * BLKW
    EPAD = WTOT * P
    maps = []
    for c in range(NC):
        ed, es, blk, q = cores[c]
        idx16 = np.zeros(EPAD, np.int16)
        dstoff = np.full(EPAD, -1.0, np.float32)
        # bucket fill
        pos = 0
        start = np.searchsorted(blk * NQ + q, np.arange(NB * NQ), side="left")
        end = np.searchsorted(blk * NQ + q, np.arange(NB * NQ), side="right")
        for b in range(NB):
            for qq in range(NQ):
                s, e = start[b * NQ + qq], end[b * NQ + qq]
                cnt = e - s
                base = (b * NQ + qq) * QW * P
                idx16[base:base + cnt] = (es[s:e] - qq * QCH).astype(np.int16)
                dstoff[base:base + cnt] = (ed[s:e] % P).astype(np.float32)
        idx_t = np.zeros((P, EPAD // 16), np.int16)
        rep = idx16.reshape(EPAD // 16, 16).T
        for r in range(8):
            idx_t[r * 16:(r + 1) * 16, :] = rep
        edst = dstoff.reshape(WTOT, P).T.copy()
        # batch ids feature-major per node tile; fake nodes -> -1
        bm = np.full(NPC, -1.0, np.float32)
        bm[:NPC_REAL] = batch[c * NPC_REAL:(c + 1) * NPC_REAL].astype(np.float32)
        bat_t = bm.reshape(NB, P).T.copy()
        maps.append({"eidx": idx_t, "edst": edst, "bat": bat_t})
    return maps, QW


def kernel(x, edge_index, batch, ggc_weight, W_ih, W_hh, b_ih, b_hh,
           mlp1_W, mlp1_b, mlp2_W, mlp2_b, out_W, out_b):
    x = np.asarray(x, np.float32)
    maps, QW = _prep(x, edge_index, batch)
    nc = _build(QW)
    # shared weights
    wiht = np.ascontiguousarray(np.asarray(W_ih, np.float32).T)      # [200, 600]
    whht = np.ascontiguousarray(np.asarray(W_hh, np.float32).T)
    b_ih = np.asarray(b_ih, np.float32)
    b_hh = np.asarray(b_hh, np.float32)
    bias4 = np.stack([
        b_ih[0:D] + b_hh[0:D],
        b_ih[D:2 * D] + b_hh[D:2 * D],
        b_ih[2 * D:3 * D],
        b_hh[2 * D:3 * D],
    ], axis=1).astype(np.float32)                                     # [200, 4]
    w1t = np.ascontiguousarray(np.asarray(mlp1_W, np.float32).T)     # [300, 256]
    w2t = np.ascontiguousarray(np.asarray(mlp2_W, np.float32).T)     # [256, 64]
    outw = np.tile(np.asarray(out_W, np.float32).reshape(1, 64), (P, 1))
    outb = np.full((P, 1), np.float32(np.asarray(out_b).reshape(-1)[0]))
    shared = {
        "wg": np.asarray(ggc_weight, np.float32),
        "wiht": wiht, "whht": whht, "bias4": bias4,
        "w1t": w1t, "b1": np.ascontiguousarray(np.asarray(mlp1_b, np.float32).reshape(2, P).T),
        "w2t": w2t, "b2": np.asarray(mlp2_b, np.float32).reshape(64, 1),
        "outw": outw, "outb": outb,
    }
    in_maps = []
    for c in range(NC):
        h0 = np.zeros((D, NPC), np.float32)
        h0[0:EFIN, 0:NPC_REAL] = x[c * NPC_REAL:(c + 1) * NPC_REAL, :].T
        xfm = np.zeros((EFIN, NPC), np.float32)
        xfm[:, 0:NPC_REAL] = x[c * NPC_REAL:(c + 1) * NPC_REAL, :].T
        m = {"h0": h0, "xfm": xfm, **maps[c], **shared}
        in_maps.append(m)
    trace = os.environ.get("GNN_TRACE") == "1"
    res = run_bass_kernel_spmd(nc, in_maps, list(range(NC)), trace=trace)
    if trace:
        print("HW exec_time_ns:", res.exec_time_ns)
        global LAST_RESULT
        LAST_RESULT = res
    return np.asarray(res.results[0]["out"]).reshape(G).astype(np.float32)

